# revision 1
# baseline (speedup 1.0000x reference)
"""Trainium2 Bass kernel for nn_Kenn_across (gnn_message_passing).

Full pipeline on 8 NeuronCores:
  phase B: MLP  across = relu(feat @ W1 + b1) @ W2 + b2   (rows sharded 8-way)
  phase C: AllGather across slices -> replicated pre[300000, 3] table
  phase D: gather pre rows for 3 index arrays (triples sharded 8-way), 3 KENN
           clause-enhancement layers, write enhanced xz rows
  phase E: AllGather enhanced xz rows; per-row winner gather (emulates
           jax/CPU scatter-set last-write-wins), select vs across, softmax.

kernel(**inputs) takes FULL inputs and returns (out, softmax(out)).
"""
import numpy as np

import concourse.bass as bass
import concourse.mybir as mybir
import concourse.tile as tile
from concourse import bacc
from concourse.bass_utils import run_bass_kernel_spmd
from concourse.masks import make_identity

# problem constants (spec: nn_Kenn_across_29661044146692)
N_CORES = 8
NW, NA, M = 100000, 200000, 262144
D = 1024
H = 1024
NT = NW + NA                 # 300000 rows in pre table
RPC = NA // N_CORES          # 25000 across rows per core
MC = M // N_CORES            # 32768 triples per core
N_LAYERS = 3
IT = 512                     # i-tile (rows of feat per inner iteration)
NTILES = (RPC + IT - 1) // IT   # 49 (48 full + 424)
DC = D // 128                # 8 contraction chunks
HC = H // 128                # 8 hidden chunks
KG = MC // 128               # 256 gather slots per partition
PS = (RPC + 127) // 128      # 196 slots per partition for across rows
RPAD = 128 * PS              # 25088

f32 = mybir.dt.float32
bf16 = mybir.dt.bfloat16
i32 = mybir.dt.int32
AF = mybir.ActivationFunctionType
ALU = mybir.AluOpType

_CACHE = {}


def _build():
    nc = bacc.Bacc("TRN2", target_bir_lowering=False, debug=False,
                   num_devices=N_CORES)

    feat = nc.dram_tensor("feat", [RPC, D], f32, kind="ExternalInput")
    w1 = nc.dram_tensor("w1", [D, H], f32, kind="ExternalInput")
    b1 = nc.dram_tensor("b1", [H], f32, kind="ExternalInput")
    w2 = nc.dram_tensor("w2", [H, 3], f32, kind="ExternalInput")
    b2 = nc.dram_tensor("b2", [3, 1], f32, kind="ExternalInput")
    win3 = nc.dram_tensor("win3", [NW, 3], f32, kind="ExternalInput")
    ixy = nc.dram_tensor("ixy", [RPAD], i32, kind="ExternalInput")
    iyz = nc.dram_tensor("iyz", [RPAD], i32, kind="ExternalInput")
    ixz = nc.dram_tensor("ixz", [RPAD], i32, kind="ExternalInput")
    cw = nc.dram_tensor("cw", [128, N_LAYERS * 3], f32, kind="ExternalInput")
    pmask = nc.dram_tensor("pmask", [128, PS], mybir.dt.int8, kind="ExternalInput")

    out_o = nc.dram_tensor("out_o", [RPC, 3], f32, kind="ExternalOutput")
    sm_o = nc.dram_tensor("sm_o", [RPC, 3], f32, kind="ExternalOutput")

    with tile.TileContext(nc) as tc:
        with tc.tile_pool(name="const", bufs=1) as cp, \
             tc.tile_pool(name="work", bufs=2) as wp, \
             tc.tile_pool(name="psT", bufs=2, space="PSUM") as ppT, \
             tc.tile_pool(name="ps1", bufs=2, space="PSUM") as pp1, \
             tc.tile_pool(name="ps2", bufs=2, space="PSUM") as pp2, \
             tc.tile_pool(name="dram", bufs=1, space="DRAM") as dp:

            # ---------- phase A: constants ----------
            idt = cp.tile([128, 128], bf16)
            make_identity(nc, idt)

            w1bf = cp.tile([128, DC, H], bf16)
            for dc in range(DC):
                nc.gpsimd.dma_start(w1bf[:, dc, :], w1[dc * 128:(dc + 1) * 128, :])
            w2bf = cp.tile([128, HC, 3], bf16)
            nc.gpsimd.dma_start(w2bf[:], w2[:].rearrange("(hc p) c -> p hc c", p=128))
            b1sb = cp.tile([128, HC], f32)
            nc.sync.dma_start(b1sb[:], b1[:].rearrange("(hc p) -> p hc", p=128))
            b2sb = cp.tile([3, 1], f32)
            nc.sync.dma_start(b2sb[:], b2[:])
            cwsb = cp.tile([128, 1, N_LAYERS * 3], f32)
            nc.sync.dma_start(cwsb[:], cw[:])

            pre = dp.tile([NT, 3], f32)
            acc_local = dp.tile([RPC, 3], f32)
            acc_full = dp.tile([NA, 3], f32, addr_space="Shared")

            nc.sync.dma_start(pre[0:NW, :], win3[:])

            # ---------- phase B: MLP ----------
            for it in range(NTILES):
                r0 = it * IT
                iw = min(IT, RPC - r0)
                nb = (iw + 127) // 128

                af = wp.tile([128, 4, D], bf16, tag="af", bufs=3)
                for b in range(nb):
                    rows = min(128, iw - b * 128)
                    nc.gpsimd.dma_start(af[:rows, b, :],
                                        feat[r0 + b * 128:r0 + b * 128 + rows, :])

                at = wp.tile([128, DC, IT], bf16, tag="at")
                for dc in range(DC):
                    pT = ppT.tile([128, 4, 128], f32, tag="pT")
                    for b in range(nb):
                        rows = min(128, iw - b * 128)
                        nc.tensor.matmul(pT[:, b, :rows],
                                         lhsT=af[:rows, b, dc * 128:(dc + 1) * 128],
                                         rhs=idt[:rows, :rows],
                                         start=True, stop=True)
                    nc.vector.tensor_copy(at[:, dc, :iw],
                                          pT[:].rearrange("p b f -> p (b f)")[:, :iw])

                c1t = wp.tile([128, HC, IT], bf16, tag="c1t")
                for hc in range(HC):
                    p1 = pp1.tile([128, IT], f32, tag="p1")
                    for dc in range(DC):
                        nc.tensor.matmul(p1[:, :iw],
                                         lhsT=w1bf[:, dc, hc * 128:(hc + 1) * 128],
                                         rhs=at[:, dc, :iw],
                                         start=(dc == 0), stop=(dc == DC - 1))
                    nc.scalar.activation(c1t[:, hc, :iw], p1[:, :iw], AF.Relu,
                                         bias=b1sb[:, hc:hc + 1], scale=1.0)

                p2 = pp2.tile([3, IT], f32, tag="p2")
                for hc in range(HC):
                    nc.tensor.matmul(p2[:, :iw],
                                     lhsT=w2bf[:, hc, :],
                                     rhs=c1t[:, hc, :iw],
                                     start=(hc == 0), stop=(hc == HC - 1))
                acc_sb = wp.tile([3, IT], f32, tag="acc", bufs=3)
                nc.scalar.activation(acc_sb[:, :iw], p2[:, :iw], AF.Identity,
                                     bias=b2sb[:, 0:1], scale=1.0)
                nc.sync.dma_start(
                    acc_local[r0:r0 + iw, :].rearrange("r c -> c r"),
                    acc_sb[:, :iw])

            # ---------- phase C: allgather across, build pre ----------
            nc.gpsimd.collective_compute(
                "AllGather", ALU.bypass,
                replica_groups=[list(range(N_CORES))],
                ins=[acc_local[:]], outs=[acc_full[:]])
            nc.sync.dma_start(pre[NW:NT, :], acc_full[:])

            # ---------- phase D: gather + KENN ----------
            u = {}
            for name, arr in (("xy", ixy), ("yz", iyz), ("xz", ixz)):
                isb = wp.tile([128, PS], i32, tag=f"i{name}", bufs=1)
                nc.sync.dma_start(isb[:], arr[:].rearrange("(p k) -> p k", p=128))
                ut = wp.tile([128, PS, 3], f32, tag=f"u{name}", bufs=1)
                for k in range(PS):
                    nc.gpsimd.indirect_dma_start(
                        out=ut[:, k, :], out_offset=None, in_=pre[:],
                        in_offset=bass.IndirectOffsetOnAxis(ap=isb[:, k:k + 1],
                                                            axis=0))
                u[name] = ut

            for l in range(N_LAYERS):
                exy = wp.tile([128, PS, 3], f32, tag="exy")
                eyz = wp.tile([128, PS, 3], f32, tag="eyz")
                exz = wp.tile([128, PS, 3], f32, tag="exz")
                nc.scalar.activation(exy[:], u["xy"][:], AF.Exp, scale=-1.0)
                nc.scalar.activation(eyz[:], u["yz"][:], AF.Exp, scale=-1.0)
                nc.scalar.activation(exz[:], u["xz"][:], AF.Exp, scale=1.0)
                ssum = wp.tile([128, PS, 3], f32, tag="ssum")
                nc.vector.tensor_tensor(ssum[:], exy[:], eyz[:], op=ALU.add)
                nc.vector.tensor_tensor(ssum[:], ssum[:], exz[:], op=ALU.add)
                nc.vector.reciprocal(ssum[:], ssum[:])
                rw = wp.tile([128, PS, 3], f32, tag="rw")
                cwb = cwsb[:, :, l * 3:(l + 1) * 3].to_broadcast([128, PS, 3])
                nc.vector.tensor_tensor(rw[:], ssum[:], cwb, op=ALU.mult)
                for name, op in (("xy", ALU.subtract), ("yz", ALU.subtract),
                                 ("xz", ALU.add)):
                    e = {"xy": exy, "yz": eyz, "xz": exz}[name]
                    nc.vector.tensor_tensor(e[:], e[:], rw[:], op=ALU.mult)
                    nc.vector.tensor_tensor(u[name][:], u[name][:], e[:], op=op)

            msb = wp.tile([128, PS, 1], mybir.dt.int8, tag="msb", bufs=1)
            nc.sync.dma_start(msb[:], pmask[:])

            acr = wp.tile([128, PS * 3], f32, tag="acr", bufs=1)
            nc.vector.memset(acr[:], 0.0)
            nc.sync.dma_start(
                acr[:127, :],
                acc_local[0:127 * PS, :].rearrange("(p s) c -> p (s c)", p=127))
            nc.sync.dma_start(
                acr[127:128, :(RPC - 127 * PS) * 3],
                acc_local[127 * PS:RPC, :].rearrange("(a s) c -> a (s c)", a=1))

            acr3 = acr[:].rearrange("p (s c) -> p s c", c=3)
            nc.vector.copy_predicated(
                acr3, msb[:].to_broadcast([128, PS, 3]), u["xz"][:])

            # softmax over c
            mx = wp.tile([128, PS], f32, tag="mx", bufs=1)
            nc.vector.tensor_tensor(mx[:], acr3[:, :, 0], acr3[:, :, 1], op=ALU.max)
            nc.vector.tensor_tensor(mx[:], mx[:], acr3[:, :, 2], op=ALU.max)
            e3 = wp.tile([128, PS * 3], f32, tag="e3", bufs=1)
            e33 = e3[:].rearrange("p (s c) -> p s c", c=3)
            for c in range(3):
                nc.vector.tensor_tensor(e33[:, :, c], acr3[:, :, c], mx[:],
                                        op=ALU.subtract)
            nc.scalar.activation(e3[:], e3[:], AF.Exp, scale=1.0)
            ssm = wp.tile([128, PS], f32, tag="ssm", bufs=1)
            nc.vector.tensor_tensor(ssm[:], e33[:, :, 0], e33[:, :, 1], op=ALU.add)
            nc.vector.tensor_tensor(ssm[:], ssm[:], e33[:, :, 2], op=ALU.add)
            nc.vector.reciprocal(ssm[:], ssm[:])
            sm = wp.tile([128, PS * 3], f32, tag="sm", bufs=1)
            sm3 = sm[:].rearrange("p (s c) -> p s c", c=3)
            for c in range(3):
                nc.vector.tensor_tensor(sm3[:, :, c], e33[:, :, c], ssm[:],
                                        op=ALU.mult)

            tail = RPC - 127 * PS
            nc.sync.dma_start(
                out_o[0:127 * PS, :].rearrange("(p s) c -> p (s c)", p=127),
                acr[:127, :])
            nc.sync.dma_start(
                out_o[127 * PS:RPC, :].rearrange("(a s) c -> a (s c)", a=1),
                acr[127:128, :tail * 3])
            nc.sync.dma_start(
                sm_o[0:127 * PS, :].rearrange("(p s) c -> p (s c)", p=127),
                sm[:127, :])
            nc.sync.dma_start(
                sm_o[127 * PS:RPC, :].rearrange("(a s) c -> a (s c)", a=1),
                sm[127:128, :tail * 3])

    nc.compile()
    return nc


def kernel(features, within_pre, index_xy, index_yz, index_xz,
           W1, b1, W2, b2, clause_weights):
    if "nc" not in _CACHE:
        _CACHE["nc"] = _build()
    nc = _CACHE["nc"]

    features = np.ascontiguousarray(features, dtype=np.float32)
    win3 = np.ascontiguousarray(within_pre[:, :3], dtype=np.float32)
    cwb = np.broadcast_to(
        np.asarray(clause_weights, np.float32).reshape(1, N_LAYERS * 3),
        (128, N_LAYERS * 3)).copy()
    b2r = np.asarray(b2, np.float32).reshape(3, 1)

    # winner permutation: emulates pre.at[index_xz].set(...) with CPU/numpy
    # last-write-wins semantics, restricted to across rows
    perm_full = np.full(NT, -1, dtype=np.int64)
    perm_full[np.asarray(index_xz)] = np.arange(M)
    perm_a = perm_full[NW:]
    mask_a = (perm_a >= 0).astype(np.int8)
    pidx_a = np.where(perm_a >= 0, perm_a, 0).astype(np.int64)
    ixy_w = np.asarray(index_xy, np.int32)[pidx_a]
    iyz_w = np.asarray(index_yz, np.int32)[pidx_a]
    ixz_w = np.asarray(index_xz, np.int32)[pidx_a]

    in_maps = []
    for k in range(N_CORES):
        sl = slice(k * RPC, (k + 1) * RPC)
        mk = np.pad(mask_a[sl], (0, RPAD - RPC)).reshape(128, PS)
        in_maps.append({
            "feat": features[sl],
            "w1": np.asarray(W1, np.float32),
            "b1": np.asarray(b1, np.float32),
            "w2": np.asarray(W2, np.float32),
            "b2": b2r,
            "win3": win3,
            "ixy": np.pad(ixy_w[sl], (0, RPAD - RPC)),
            "iyz": np.pad(iyz_w[sl], (0, RPAD - RPC)),
            "ixz": np.pad(ixz_w[sl], (0, RPAD - RPC)),
            "cw": cwb,
            "pmask": np.ascontiguousarray(mk),
        })

    res = run_bass_kernel_spmd(nc, in_maps, core_ids=list(range(N_CORES)),
                               **_CACHE.get("run_kwargs", {}))
    _CACHE["last_results"] = res
    out = np.concatenate([res.results[k]["out_o"] for k in range(N_CORES)], axis=0)
    sm = np.concatenate([res.results[k]["sm_o"] for k in range(N_CORES)], axis=0)
    return out, sm



# revision 8
# speedup vs baseline: 1.8072x; 1.8072x over previous
"""Trainium2 Bass kernel for nn_Kenn_across (gnn_message_passing).

Pipeline on 8 NeuronCores (SPMD), per core:
  MLP     across = relu(featT @ W1 + b1) @ W2 + b2 on RPAD=25088 rows,
          features host-pretransposed to bf16 [DC,128,RPAD] (no PE transposes).
  AG      chunked AllGather (4 chunks) of across slices directly into the
          replicated pre table, overlapping the MLP.
  gather  per-column [128,1] indirect DMAs for u_xy / u_yz, issued in
          dependency order (within-table columns at t=0, chunk-c columns right
          after AG_c) so almost all SWDGE time hides under the MLP.
          u_xz needs NO gather: each across row's winning triple has
          index_xz == that row, so u_xz == own across value (dense read).
  KENN    3 clause-enhancement layers (slot-wise vector/scalar ops).
  out     masked select vs across, softmax, raw [128, PS*3] outputs
          (host decodes slot order).

Host-side prep (all integer/bookkeeping, plus feature permute+bf16):
  - winner permutation per across row (last-write-wins scatter semantics)
  - slots sorted within fixed AG-chunk ranges by (xy-dep, yz-dep) class so
    gather columns are dependency-homogeneous; mask-0 slots sorted first and
    their columns skipped entirely
  - gather indices rewritten to the device table layout
"""
import hashlib
import numpy as np
import ml_dtypes

import concourse.bass as bass
import concourse.mybir as mybir
import concourse.tile as tile
from concourse import bacc
from concourse.bass_utils import run_bass_kernel_spmd

# problem constants (spec: nn_Kenn_across_29661044146692)
N_CORES = 8
NW, NA, M = 100000, 200000, 262144
D, H = 1024, 1024
RPC = NA // N_CORES              # 25000 across rows per core
PS = 196                         # columns (slots per partition)
RPAD = 128 * PS                  # 25088 padded slots per core
NT2 = NW + N_CORES * RPAD        # device pre-table rows (300704)
DC = D // 128
HC = H // 128
IT = 512                         # MLP tile rows; 49 * 512 == RPAD
NTILES = RPAD // IT
N_LAYERS = 3

# AG chunk geometry (columns per chunk; multiples of 4 so chunk ends align
# with MLP tiles). Geometric-ish: small last chunk minimizes the tail.
SC_COLS = (76, 60, 40, 20)
NCH = len(SC_COLS)
S_CUM = tuple(np.cumsum((0,) + tuple(c * 128 for c in SC_COLS)).tolist())
AG_TILES = tuple(S_CUM[c + 1] // IT - 1 for c in range(NCH))  # 18,33,43,48

f32 = mybir.dt.float32
bf16 = mybir.dt.bfloat16
i32 = mybir.dt.int32
i8 = mybir.dt.int8
AF = mybir.ActivationFunctionType
ALU = mybir.AluOpType

_CACHE = {}


def _build(gather_plan):
    """gather_plan: list of (ub_rows, name, col) in issue order; name in
    {"xy", "yz"}; the indirect gather for that column reads pre[0:ub_rows]."""
    nc = bacc.Bacc("TRN2", target_bir_lowering=False, debug=False,
                   num_devices=N_CORES)

    featT = nc.dram_tensor("featT", [DC, 128, RPAD], bf16, kind="ExternalInput")
    w1b = nc.dram_tensor("w1b", [DC, 128, H], bf16, kind="ExternalInput")
    w2b = nc.dram_tensor("w2b", [HC, 128, 3], bf16, kind="ExternalInput")
    b1 = nc.dram_tensor("b1", [H], f32, kind="ExternalInput")
    b2 = nc.dram_tensor("b2", [3, 1], f32, kind="ExternalInput")
    win3 = nc.dram_tensor("win3", [NW, 3], f32, kind="ExternalInput")
    cw = nc.dram_tensor("cw", [128, N_LAYERS * 3], f32, kind="ExternalInput")
    isx = nc.dram_tensor("isx", [128, PS], i32, kind="ExternalInput")
    isy = nc.dram_tensor("isy", [128, PS], i32, kind="ExternalInput")
    pmask = nc.dram_tensor("pmask", [128, PS], i8, kind="ExternalInput")

    out_o = nc.dram_tensor("out_o", [128, PS * 3], f32, kind="ExternalOutput")
    sm_o = nc.dram_tensor("sm_o", [128, PS * 3], f32, kind="ExternalOutput")

    with tile.TileContext(nc) as tc:
        with tc.tile_pool(name="const", bufs=1) as cp, \
             tc.tile_pool(name="work", bufs=2) as wp, \
             tc.tile_pool(name="ps1", bufs=2, space="PSUM") as pp1, \
             tc.tile_pool(name="ps2", bufs=2, space="PSUM") as pp2, \
             tc.tile_pool(name="dram", bufs=1, space="DRAM") as dp:

            # ---------- constants ----------
            w1sb = cp.tile([128, DC, H], bf16)
            nc.sync.dma_start(w1sb[:], w1b[:].rearrange("d p h -> p d h"))
            w2sb = cp.tile([128, HC, 3], bf16)
            nc.sync.dma_start(w2sb[:], w2b[:].rearrange("h p c -> p h c"))
            b1sb = cp.tile([128, HC], f32)
            nc.sync.dma_start(b1sb[:], b1[:].rearrange("(hc p) -> p hc", p=128))
            b2sb = cp.tile([3, 1], f32)
            nc.sync.dma_start(b2sb[:], b2[:])
            cwsb = cp.tile([128, 1, N_LAYERS * 3], f32)
            nc.sync.dma_start(cwsb[:], cw[:])
            isxsb = cp.tile([128, PS], i32)
            nc.sync.dma_start(isxsb[:], isx[:])
            isysb = cp.tile([128, PS], i32)
            nc.sync.dma_start(isysb[:], isy[:])
            msb = cp.tile([128, PS, 1], i8)
            nc.sync.dma_start(msb[:], pmask[:])

            pre = dp.tile([NT2, 3], f32)
            acc = dp.tile([RPAD, 3], f32)
            agt = []
            for c in range(NCH):
                agt_c = dp.tile([N_CORES * (S_CUM[c + 1] - S_CUM[c]), 3], f32,
                                addr_space="Shared", name=f"agt{c}")
                agt.append(agt_c)
            nc.sync.dma_start(pre[0:NW, :], win3[:])

            # u tiles; xy/yz filled by gathers (issued below, interleaved
            # with the MLP in program order; Tile range-deps do the timing)
            uxy = wp.tile([128, PS, 3], f32, tag="uxy", bufs=1)
            uyz = wp.tile([128, PS, 3], f32, tag="uyz", bufs=1)
            nc.vector.memset(uxy[:], 0.0)
            nc.vector.memset(uyz[:], 0.0)

            # ---------- gathers (program order = dependency order) ----------
            def issue_gathers(lo, hi):
                for ub, name, s in gather_plan[lo:hi]:
                    t_ = uxy if name == "xy" else uyz
                    sb_ = isxsb if name == "xy" else isysb
                    nc.gpsimd.indirect_dma_start(
                        out=t_[:, s, :], out_offset=None, in_=pre[0:ub, :],
                        in_offset=bass.IndirectOffsetOnAxis(
                            ap=sb_[:, s:s + 1], axis=0))

            # plan is sorted by ub; cuts[c] = #entries ready before AG chunk c
            cuts = [sum(1 for ub, _, _ in gather_plan if ub <= NW)]
            for c in range(NCH):
                ubc = NW + N_CORES * S_CUM[c + 1]
                cuts.append(sum(1 for ub, _, _ in gather_plan if ub <= ubc))

            issue_gathers(0, cuts[0])          # within-table columns

            # ---------- MLP ----------
            ag_chunk = 0
            for t in range(NTILES):
                r0 = t * IT
                at = wp.tile([128, DC, IT], bf16, tag="at", bufs=3)
                nc.sync.dma_start(
                    at[:], featT[:, :, r0:r0 + IT].rearrange("d p r -> p d r"))
                c1t = wp.tile([128, HC, IT], bf16, tag="c1t")
                for hc in range(HC):
                    p1 = pp1.tile([128, IT], f32, tag="p1")
                    for dc in range(DC):
                        nc.tensor.matmul(p1[:],
                                         lhsT=w1sb[:, dc, hc * 128:(hc + 1) * 128],
                                         rhs=at[:, dc, :],
                                         start=(dc == 0), stop=(dc == DC - 1))
                    nc.scalar.activation(c1t[:, hc, :], p1[:], AF.Relu,
                                         bias=b1sb[:, hc:hc + 1], scale=1.0)
                p2 = pp2.tile([3, IT], f32, tag="p2")
                for hc in range(HC):
                    nc.tensor.matmul(p2[:], lhsT=w2sb[:, hc, :],
                                     rhs=c1t[:, hc, :],
                                     start=(hc == 0), stop=(hc == HC - 1))
                acc_sb = wp.tile([3, IT], f32, tag="acc", bufs=3)
                nc.scalar.activation(acc_sb[:], p2[:], AF.Identity,
                                     bias=b2sb[:, 0:1], scale=1.0)
                nc.sync.dma_start(
                    acc[r0:r0 + IT, :].rearrange("r c -> c r"), acc_sb[:])

                if ag_chunk < NCH and t == AG_TILES[ag_chunk]:
                    c = ag_chunk
                    nc.gpsimd.collective_compute(
                        "AllGather", ALU.bypass,
                        replica_groups=[list(range(N_CORES))],
                        ins=[acc[S_CUM[c]:S_CUM[c + 1], :]],
                        outs=[agt[c][:]])
                    nc.sync.dma_start(
                        pre[NW + N_CORES * S_CUM[c]:
                            NW + N_CORES * S_CUM[c + 1], :], agt[c][:])
                    issue_gathers(cuts[c], cuts[c + 1])
                    ag_chunk += 1

            # ---------- u_xz: own across values, dense ----------
            uxz = wp.tile([128, PS, 3], f32, tag="uxz", bufs=1)
            nc.sync.dma_start(
                uxz[:], acc[:].rearrange("(s p) c -> p s c", p=128))

            # ---------- KENN layers ----------
            u = {"xy": uxy, "yz": uyz, "xz": uxz}
            for l in range(N_LAYERS):
                exy = wp.tile([128, PS, 3], f32, tag="exy")
                eyz = wp.tile([128, PS, 3], f32, tag="eyz")
                exz = wp.tile([128, PS, 3], f32, tag="exz")
                nc.scalar.activation(exy[:], u["xy"][:], AF.Exp, scale=-1.0)
                nc.scalar.activation(eyz[:], u["yz"][:], AF.Exp, scale=-1.0)
                nc.scalar.activation(exz[:], u["xz"][:], AF.Exp, scale=1.0)
                ssum = wp.tile([128, PS, 3], f32, tag="ssum")
                nc.vector.tensor_tensor(ssum[:], exy[:], eyz[:], op=ALU.add)
                nc.vector.tensor_tensor(ssum[:], ssum[:], exz[:], op=ALU.add)
                nc.vector.reciprocal(ssum[:], ssum[:])
                rw = wp.tile([128, PS, 3], f32, tag="rw")
                cwb = cwsb[:, :, l * 3:(l + 1) * 3].to_broadcast([128, PS, 3])
                nc.vector.tensor_tensor(rw[:], ssum[:], cwb, op=ALU.mult)
                for name, op in (("xy", ALU.subtract), ("yz", ALU.subtract),
                                 ("xz", ALU.add)):
                    e = {"xy": exy, "yz": eyz, "xz": exz}[name]
                    nc.vector.tensor_tensor(e[:], e[:], rw[:], op=ALU.mult)
                    nc.vector.tensor_tensor(u[name][:], u[name][:], e[:], op=op)

            # ---------- select + softmax + outputs ----------
            acr = wp.tile([128, PS, 3], f32, tag="acr", bufs=1)
            nc.sync.dma_start(
                acr[:], acc[:].rearrange("(s p) c -> p s c", p=128))
            nc.vector.copy_predicated(
                acr[:], msb[:].to_broadcast([128, PS, 3]), u["xz"][:])

            mx = wp.tile([128, PS], f32, tag="mx", bufs=1)
            nc.vector.tensor_tensor(mx[:], acr[:, :, 0], acr[:, :, 1], op=ALU.max)
            nc.vector.tensor_tensor(mx[:], mx[:], acr[:, :, 2], op=ALU.max)
            e3 = wp.tile([128, PS, 3], f32, tag="e3", bufs=1)
            for c in range(3):
                nc.vector.tensor_tensor(e3[:, :, c], acr[:, :, c], mx[:],
                                        op=ALU.subtract)
            nc.scalar.activation(e3[:], e3[:], AF.Exp, scale=1.0)
            ssm = wp.tile([128, PS], f32, tag="ssm", bufs=1)
            nc.vector.tensor_tensor(ssm[:], e3[:, :, 0], e3[:, :, 1], op=ALU.add)
            nc.vector.tensor_tensor(ssm[:], ssm[:], e3[:, :, 2], op=ALU.add)
            nc.vector.reciprocal(ssm[:], ssm[:])
            sm = wp.tile([128, PS, 3], f32, tag="sm", bufs=1)
            for c in range(3):
                nc.vector.tensor_tensor(sm[:, :, c], e3[:, :, c], ssm[:],
                                        op=ALU.mult)

            nc.sync.dma_start(out_o[:], acr[:].rearrange("p s c -> p (s c)"))
            nc.sync.dma_start(sm_o[:], sm[:].rearrange("p s c -> p (s c)"))

    nc.compile()
    return nc


def kernel(features, within_pre, index_xy, index_yz, index_xz,
           W1, b1, W2, b2, clause_weights):
    features = np.asarray(features)
    within_pre = np.asarray(within_pre)
    index_xy = np.asarray(index_xy, np.int64)
    index_yz = np.asarray(index_yz, np.int64)
    index_xz = np.asarray(index_xz, np.int64)

    NT0 = NW + NA
    # winner per across row (numpy last-write-wins scatter semantics)
    perm_full = np.full(NT0, -1, np.int64)
    perm_full[index_xz] = np.arange(M)
    perm_a = perm_full[NW:]
    mask_a = perm_a >= 0
    pidx_a = np.where(mask_a, perm_a, 0)
    ixy_w = index_xy[pidx_a]          # per global across row j: winner's xy idx
    iyz_w = index_yz[pidx_a]

    # fixed chunk assignment of local row r (original order)
    s_cum = np.asarray(S_CUM, np.int64)
    chunk_of_local = np.searchsorted(s_cum[1:], np.arange(RPC), side="right")

    # dep class per global across row for each name:
    #   -2 no need (mask 0), -1 within-table, 0..NCH-1 referenced AG chunk
    def dep_class(idx_w):
        is_w = idx_w < NW
        rloc = (idx_w - NW) % RPC
        cls = np.where(is_w, -1, chunk_of_local[np.clip(rloc, 0, RPC - 1)])
        return np.where(mask_a, cls, -2).astype(np.int64)

    cx = dep_class(ixy_w)
    cy = dep_class(iyz_w)

    # per-core within-chunk sort by (cx, cy); slot_of_row / row_at_slot
    row_at_slot = np.full((N_CORES, RPAD), -1, np.int64)   # local row ids
    key_all = (cx + 2) * (NCH + 2) + (cy + 2)
    for k in range(N_CORES):
        keys = key_all[k * RPC:(k + 1) * RPC]
        for c in range(NCH):
            lo, hi = int(s_cum[c]), min(int(s_cum[c + 1]), RPC)
            order = np.argsort(keys[lo:hi], kind="stable") + lo
            row_at_slot[k, lo:lo + (hi - lo)] = order
    slot_of_row = np.full((N_CORES, RPC), -1, np.int64)
    for k in range(N_CORES):
        valid = row_at_slot[k] >= 0
        slot_of_row[k, row_at_slot[k, valid]] = np.nonzero(valid)[0]

    # device-table position of original across index a (global, 0..NA)
    def table_pos(a):
        k2 = a // RPC
        r2 = a % RPC
        c2 = chunk_of_local[r2]
        l2 = slot_of_row[k2, r2]
        return (NW + N_CORES * s_cum[c2] + k2 * (s_cum[c2 + 1] - s_cum[c2])
                + (l2 - s_cum[c2]))

    # rewritten per-core per-slot gather indices + per-column deps
    isx_np = np.zeros((N_CORES, 128, PS), np.int32)
    isy_np = np.zeros((N_CORES, 128, PS), np.int32)
    msk_np = np.zeros((N_CORES, 128, PS), np.int8)
    plans = None
    for k in range(N_CORES):
        ras = row_at_slot[k]
        valid = ras >= 0
        g = k * RPC + np.where(valid, ras, 0)
        m_slot = np.where(valid, mask_a[g], False)
        cx_s = np.where(m_slot, cx[g], -2)
        cy_s = np.where(m_slot, cy[g], -2)

        def rewrite(idx_w):
            i0 = idx_w[g]
            pos = np.where(i0 < NW, i0, 0)
            a = np.where(i0 >= NW, i0 - NW, 0)
            posA = table_pos(a)
            out = np.where(i0 < NW, pos, posA)
            return np.where(m_slot, out, 0).astype(np.int32)

        ix = rewrite(ixy_w)
        iy = rewrite(iyz_w)
        # slot l = s*128 + p  ->  isb[p, s]
        isx_np[k] = ix.reshape(PS, 128).T
        isy_np[k] = iy.reshape(PS, 128).T
        msk_np[k] = m_slot.astype(np.int8).reshape(PS, 128).T

        if k == 0:
            # build the gather plan from core 0 (identical structure enforced
            # by taking the max dep over all cores below)
            plans = [cx_s.reshape(PS, 128), cy_s.reshape(PS, 128)]
        else:
            plans[0] = np.maximum(plans[0], cx_s.reshape(PS, 128))
            plans[1] = np.maximum(plans[1], cy_s.reshape(PS, 128))

    gather_plan = []
    for name, cls_cols in (("xy", plans[0]), ("yz", plans[1])):
        col_dep = cls_cols.max(axis=1)          # [PS]
        for s in range(PS):
            d = int(col_dep[s])
            if d == -2:
                continue                         # no slot needs this column
            ub = NW if d == -1 else NW + N_CORES * int(s_cum[d + 1])
            gather_plan.append((ub, name, s))
    gather_plan.sort(key=lambda e: e[0])

    plan_key = hashlib.sha256(repr(gather_plan).encode()).hexdigest()
    if _CACHE.get("plan_key") != plan_key:
        _CACHE["nc"] = _build(gather_plan)
        _CACHE["plan_key"] = plan_key
    nc = _CACHE["nc"]

    # ---------- numeric inputs ----------
    w1bf = np.ascontiguousarray(
        np.asarray(W1, np.float32).reshape(DC, 128, H)).astype(ml_dtypes.bfloat16)
    w2bf = np.ascontiguousarray(
        np.asarray(W2, np.float32).reshape(HC, 128, 3)).astype(ml_dtypes.bfloat16)
    b1f = np.asarray(b1, np.float32)
    b2r = np.asarray(b2, np.float32).reshape(3, 1)
    win3 = np.ascontiguousarray(within_pre[:, :3], np.float32)
    cwb = np.broadcast_to(
        np.asarray(clause_weights, np.float32).reshape(1, N_LAYERS * 3),
        (128, N_LAYERS * 3)).copy()

    in_maps = []
    for k in range(N_CORES):
        ras = row_at_slot[k]
        src = k * RPC + np.where(ras >= 0, ras, 0)
        fp = features[src].astype(ml_dtypes.bfloat16)
        fp[ras < 0] = 0
        featT_k = np.ascontiguousarray(fp.T.reshape(DC, 128, RPAD))
        in_maps.append({
            "featT": featT_k,
            "w1b": w1bf, "w2b": w2bf, "b1": b1f, "b2": b2r,
            "win3": win3, "cw": cwb,
            "isx": isx_np[k], "isy": isy_np[k], "pmask": msk_np[k],
        })

    res = run_bass_kernel_spmd(nc, in_maps, core_ids=list(range(N_CORES)))
    _CACHE["last_results"] = res

    out = np.empty((NA, 3), np.float32)
    smx = np.empty((NA, 3), np.float32)
    for k in range(N_CORES):
        raw_o = res.results[k]["out_o"].reshape(128, PS, 3)
        raw_s = res.results[k]["sm_o"].reshape(128, PS, 3)
        o_slot = raw_o.transpose(1, 0, 2).reshape(RPAD, 3)
        s_slot = raw_s.transpose(1, 0, 2).reshape(RPAD, 3)
        ras = row_at_slot[k]
        valid = ras >= 0
        out[k * RPC + ras[valid]] = o_slot[valid]
        smx[k * RPC + ras[valid]] = s_slot[valid]
    return out, smx


# revision 12
# speedup vs baseline: 1.8415x; 1.0190x over previous
"""Trainium2 Bass kernel for nn_Kenn_across (gnn_message_passing).

Pipeline on 8 NeuronCores (SPMD), per core:
  MLP     across = relu(featT @ W1 + b1) @ W2 + b2 on RPAD=25088 rows,
          features host-pretransposed to bf16 [DC,128,RPAD] (no PE transposes).
  AG      chunked AllGather (4 chunks) of across slices directly into the
          replicated pre table, overlapping the MLP.
  gather  per-column [128,1] indirect DMAs for u_xy / u_yz, issued in
          dependency order (within-table columns at t=0, chunk-c columns right
          after AG_c) so almost all SWDGE time hides under the MLP.
          u_xz needs NO gather: each across row's winning triple has
          index_xz == that row, so u_xz == own across value (dense read).
  KENN    3 clause-enhancement layers (slot-wise vector/scalar ops).
  out     masked select vs across, softmax, raw [128, PS*3] outputs
          (host decodes slot order).

Host-side prep (all integer/bookkeeping, plus feature permute+bf16):
  - winner permutation per across row (last-write-wins scatter semantics)
  - slots sorted within fixed AG-chunk ranges by (xy-dep, yz-dep) class so
    gather columns are dependency-homogeneous; mask-0 slots sorted first and
    their columns skipped entirely
  - gather indices rewritten to the device table layout
"""
import hashlib
import numpy as np
import ml_dtypes

import concourse.bass as bass
import concourse.mybir as mybir
import concourse.tile as tile
from concourse import bacc
from concourse.bass_utils import run_bass_kernel_spmd

# problem constants (spec: nn_Kenn_across_29661044146692)
N_CORES = 8
NW, NA, M = 100000, 200000, 262144
D, H = 1024, 1024
RPC = NA // N_CORES              # 25000 across rows per core
PS = 196                         # columns (slots per partition)
RPAD = 128 * PS                  # 25088 padded slots per core
NT2 = NW + N_CORES * RPAD        # device pre-table rows (300704)
DC = D // 128
HC = H // 128
IT = 512                         # MLP tile rows; 49 * 512 == RPAD
NTILES = RPAD // IT
N_LAYERS = 3

# AG chunk geometry (columns per chunk; multiples of 4 so chunk ends align
# with MLP tiles). Geometric-ish: small last chunk minimizes the tail.
SC_COLS = (76, 60, 44, 16)
NCH = len(SC_COLS)
S_CUM = tuple(np.cumsum((0,) + tuple(c * 128 for c in SC_COLS)).tolist())
AG_TILES = tuple(S_CUM[c + 1] // IT - 1 for c in range(NCH))  # 18,33,43,48

f32 = mybir.dt.float32
bf16 = mybir.dt.bfloat16
i32 = mybir.dt.int32
i8 = mybir.dt.int8
AF = mybir.ActivationFunctionType
ALU = mybir.AluOpType

_CACHE = {}


def _build(gather_plan):
    """gather_plan: list of (ub_rows, name, col) in issue order; name in
    {"xy", "yz"}; the indirect gather for that column reads pre[0:ub_rows]."""
    nc = bacc.Bacc("TRN2", target_bir_lowering=False, debug=False,
                   num_devices=N_CORES)

    featT = nc.dram_tensor("featT", [DC, 128, RPAD], bf16, kind="ExternalInput")
    w1b = nc.dram_tensor("w1b", [DC, 128, H], bf16, kind="ExternalInput")
    w2b = nc.dram_tensor("w2b", [HC, 128, 3], bf16, kind="ExternalInput")
    b1 = nc.dram_tensor("b1", [H], f32, kind="ExternalInput")
    b2 = nc.dram_tensor("b2", [3, 1], f32, kind="ExternalInput")
    win3 = nc.dram_tensor("win3", [NW, 3], f32, kind="ExternalInput")
    cw = nc.dram_tensor("cw", [128, N_LAYERS * 3], f32, kind="ExternalInput")
    isx = nc.dram_tensor("isx", [128, PS], i32, kind="ExternalInput")
    isy = nc.dram_tensor("isy", [128, PS], i32, kind="ExternalInput")
    pmask = nc.dram_tensor("pmask", [128, PS], i8, kind="ExternalInput")

    out_o = nc.dram_tensor("out_o", [128, PS * 3], f32, kind="ExternalOutput")
    sm_o = nc.dram_tensor("sm_o", [128, PS * 3], f32, kind="ExternalOutput")

    with tile.TileContext(nc) as tc:
        with tc.tile_pool(name="const", bufs=1) as cp, \
             tc.tile_pool(name="work", bufs=2) as wp, \
             tc.tile_pool(name="ps1", bufs=2, space="PSUM") as pp1, \
             tc.tile_pool(name="ps2", bufs=2, space="PSUM") as pp2, \
             tc.tile_pool(name="dram", bufs=1, space="DRAM") as dp:

            # ---------- constants ----------
            w1sb = cp.tile([128, DC, H], bf16)
            nc.sync.dma_start(w1sb[:], w1b[:].rearrange("d p h -> p d h"))
            w2sb = cp.tile([128, HC, 3], bf16)
            nc.sync.dma_start(w2sb[:], w2b[:].rearrange("h p c -> p h c"))
            b1sb = cp.tile([128, HC], f32)
            nc.sync.dma_start(b1sb[:], b1[:].rearrange("(hc p) -> p hc", p=128))
            b2sb = cp.tile([3, 1], f32)
            nc.sync.dma_start(b2sb[:], b2[:])
            cwsb = cp.tile([128, 1, N_LAYERS * 3], f32)
            nc.sync.dma_start(cwsb[:], cw[:])
            isxsb = cp.tile([128, PS], i32)
            nc.sync.dma_start(isxsb[:], isx[:])
            isysb = cp.tile([128, PS], i32)
            nc.sync.dma_start(isysb[:], isy[:])
            msb = cp.tile([128, PS, 1], i8)
            nc.sync.dma_start(msb[:], pmask[:])

            pre = dp.tile([NT2, 3], f32)
            acc = dp.tile([RPAD, 3], f32)
            agt = []
            for c in range(NCH):
                agt_c = dp.tile([N_CORES * (S_CUM[c + 1] - S_CUM[c]), 3], f32,
                                addr_space="Shared", name=f"agt{c}")
                agt.append(agt_c)
            nc.sync.dma_start(pre[0:NW, :], win3[:])

            # u tiles; xy/yz filled by gathers (issued below, interleaved
            # with the MLP in program order; Tile range-deps do the timing)
            uxy = wp.tile([128, PS, 3], f32, tag="uxy", bufs=1)
            uyz = wp.tile([128, PS, 3], f32, tag="uyz", bufs=1)
            nc.vector.memset(uxy[:], 0.0)
            nc.vector.memset(uyz[:], 0.0)

            # ---------- gathers (program order = dependency order) ----------
            def issue_gathers(lo, hi):
                for ub, name, s in gather_plan[lo:hi]:
                    t_ = uxy if name == "xy" else uyz
                    sb_ = isxsb if name == "xy" else isysb
                    nc.gpsimd.indirect_dma_start(
                        out=t_[:, s, :], out_offset=None, in_=pre[0:ub, :],
                        in_offset=bass.IndirectOffsetOnAxis(
                            ap=sb_[:, s:s + 1], axis=0))

            # plan is sorted by ub; cuts[c] = #entries ready before AG chunk c
            cuts = [sum(1 for ub, _, _ in gather_plan if ub <= NW)]
            for c in range(NCH):
                ubc = NW + N_CORES * S_CUM[c + 1]
                cuts.append(sum(1 for ub, _, _ in gather_plan if ub <= ubc))

            issue_gathers(0, cuts[0])          # within-table columns

            # ---------- MLP ----------
            ag_chunk = 0
            for t in range(NTILES):
                r0 = t * IT
                at = wp.tile([128, DC, IT], bf16, tag="at", bufs=3)
                nc.sync.dma_start(
                    at[:], featT[:, :, r0:r0 + IT].rearrange("d p r -> p d r"))
                c1t = wp.tile([128, HC, IT], bf16, tag="c1t")
                for hc in range(HC):
                    p1 = pp1.tile([128, IT], f32, tag="p1")
                    for dc in range(DC):
                        nc.tensor.matmul(p1[:],
                                         lhsT=w1sb[:, dc, hc * 128:(hc + 1) * 128],
                                         rhs=at[:, dc, :],
                                         start=(dc == 0), stop=(dc == DC - 1))
                    nc.scalar.activation(c1t[:, hc, :], p1[:], AF.Relu,
                                         bias=b1sb[:, hc:hc + 1], scale=1.0)
                p2 = pp2.tile([3, IT], f32, tag="p2")
                for hc in range(HC):
                    nc.tensor.matmul(p2[:], lhsT=w2sb[:, hc, :],
                                     rhs=c1t[:, hc, :],
                                     start=(hc == 0), stop=(hc == HC - 1))
                acc_sb = wp.tile([3, IT], f32, tag="acc", bufs=3)
                nc.scalar.activation(acc_sb[:], p2[:], AF.Identity,
                                     bias=b2sb[:, 0:1], scale=1.0)
                nc.sync.dma_start(
                    acc[r0:r0 + IT, :].rearrange("r c -> c r"), acc_sb[:])

                if ag_chunk < NCH and t == AG_TILES[ag_chunk]:
                    c = ag_chunk
                    nc.gpsimd.collective_compute(
                        "AllGather", ALU.bypass,
                        replica_groups=[list(range(N_CORES))],
                        ins=[acc[S_CUM[c]:S_CUM[c + 1], :]],
                        outs=[agt[c][:]])
                    nc.sync.dma_start(
                        pre[NW + N_CORES * S_CUM[c]:
                            NW + N_CORES * S_CUM[c + 1], :], agt[c][:])
                    issue_gathers(cuts[c], cuts[c + 1])
                    ag_chunk += 1

            # ---------- u_xz: own across values, dense ----------
            uxz = wp.tile([128, PS, 3], f32, tag="uxz", bufs=1)
            nc.sync.dma_start(
                uxz[:], acc[:].rearrange("(s p) c -> p s c", p=128))

            # ---------- KENN layers ----------
            u = {"xy": uxy, "yz": uyz, "xz": uxz}
            for l in range(N_LAYERS):
                exy = wp.tile([128, PS, 3], f32, tag="exy")
                eyz = wp.tile([128, PS, 3], f32, tag="eyz")
                exz = wp.tile([128, PS, 3], f32, tag="exz")
                nc.scalar.activation(exy[:], u["xy"][:], AF.Exp, scale=-1.0)
                nc.scalar.activation(eyz[:], u["yz"][:], AF.Exp, scale=-1.0)
                nc.scalar.activation(exz[:], u["xz"][:], AF.Exp, scale=1.0)
                ssum = wp.tile([128, PS, 3], f32, tag="ssum")
                nc.vector.tensor_tensor(ssum[:], exy[:], eyz[:], op=ALU.add)
                nc.vector.tensor_tensor(ssum[:], ssum[:], exz[:], op=ALU.add)
                nc.vector.reciprocal(ssum[:], ssum[:])
                rw = wp.tile([128, PS, 3], f32, tag="rw")
                cwb = cwsb[:, :, l * 3:(l + 1) * 3].to_broadcast([128, PS, 3])
                nc.vector.tensor_tensor(rw[:], ssum[:], cwb, op=ALU.mult)
                for name, op in (("xy", ALU.subtract), ("yz", ALU.subtract),
                                 ("xz", ALU.add)):
                    e = {"xy": exy, "yz": eyz, "xz": exz}[name]
                    nc.vector.tensor_tensor(e[:], e[:], rw[:], op=ALU.mult)
                    nc.vector.tensor_tensor(u[name][:], u[name][:], e[:], op=op)

            # ---------- select + softmax + outputs ----------
            acr = wp.tile([128, PS, 3], f32, tag="acr", bufs=1)
            nc.sync.dma_start(
                acr[:], acc[:].rearrange("(s p) c -> p s c", p=128))
            nc.vector.copy_predicated(
                acr[:], msb[:].to_broadcast([128, PS, 3]), u["xz"][:])

            mx = wp.tile([128, PS], f32, tag="mx", bufs=1)
            nc.vector.tensor_tensor(mx[:], acr[:, :, 0], acr[:, :, 1], op=ALU.max)
            nc.vector.tensor_tensor(mx[:], mx[:], acr[:, :, 2], op=ALU.max)
            e3 = wp.tile([128, PS, 3], f32, tag="e3", bufs=1)
            for c in range(3):
                nc.vector.tensor_tensor(e3[:, :, c], acr[:, :, c], mx[:],
                                        op=ALU.subtract)
            nc.scalar.activation(e3[:], e3[:], AF.Exp, scale=1.0)
            ssm = wp.tile([128, PS], f32, tag="ssm", bufs=1)
            nc.vector.tensor_tensor(ssm[:], e3[:, :, 0], e3[:, :, 1], op=ALU.add)
            nc.vector.tensor_tensor(ssm[:], ssm[:], e3[:, :, 2], op=ALU.add)
            nc.vector.reciprocal(ssm[:], ssm[:])
            sm = wp.tile([128, PS, 3], f32, tag="sm", bufs=1)
            for c in range(3):
                nc.vector.tensor_tensor(sm[:, :, c], e3[:, :, c], ssm[:],
                                        op=ALU.mult)

            nc.sync.dma_start(out_o[:], acr[:].rearrange("p s c -> p (s c)"))
            nc.sync.dma_start(sm_o[:], sm[:].rearrange("p s c -> p (s c)"))

    nc.compile()
    return nc


def kernel(features, within_pre, index_xy, index_yz, index_xz,
           W1, b1, W2, b2, clause_weights):
    features = np.asarray(features)
    within_pre = np.asarray(within_pre)
    index_xy = np.asarray(index_xy, np.int64)
    index_yz = np.asarray(index_yz, np.int64)
    index_xz = np.asarray(index_xz, np.int64)

    NT0 = NW + NA
    # winner per across row (numpy last-write-wins scatter semantics)
    perm_full = np.full(NT0, -1, np.int64)
    perm_full[index_xz] = np.arange(M)
    perm_a = perm_full[NW:]
    mask_a = perm_a >= 0
    pidx_a = np.where(mask_a, perm_a, 0)
    ixy_w = index_xy[pidx_a]          # per global across row j: winner's xy idx
    iyz_w = index_yz[pidx_a]

    # fixed global chunk assignment: chunk c holds original across rows
    # [G[c], G[c+1]); capacities are 8*SC_c slots (pads live in the last chunk)
    s_cum = np.asarray(S_CUM, np.int64)
    cap = np.asarray([N_CORES * (S_CUM[c + 1] - S_CUM[c]) for c in range(NCH)],
                     np.int64)
    G = np.concatenate([[0], np.cumsum(cap)])
    G = np.minimum(G, NA)

    def chunk_of_global(a):
        return np.searchsorted(G[1:], a, side="right")

    # dep class per global across row for each name:
    #   -2 no need (mask 0), -1 within-table, 0..NCH-1 referenced AG chunk
    def dep_class(idx_w):
        is_w = idx_w < NW
        cls = np.where(is_w, -1, chunk_of_global(np.maximum(idx_w - NW, 0)))
        return np.where(mask_a, cls, -2).astype(np.int64)

    cx = dep_class(ixy_w)
    cy = dep_class(iyz_w)

    # global within-chunk sort by (cx, cy), then deal column-synchronized:
    # deal position idx -> (global col idx//1024, core (idx%1024)//128,
    # partition idx%128). Gather-column classes are identical on all cores.
    key_all = (cx + 2) * (NCH + 2) + (cy + 2)
    row_at_slot_g = np.full((N_CORES, RPAD), -1, np.int64)  # global row ids
    pos_of_global = np.full(NA, -1, np.int64)               # device table row
    plan_cols = {"xy": np.full(PS, -2, np.int64),
                 "yz": np.full(PS, -2, np.int64)}
    for c in range(NCH):
        lo, hi = int(G[c]), int(G[c + 1])
        order = np.argsort(key_all[lo:hi], kind="stable") + lo   # global rows
        L = np.concatenate([order, np.full(int(cap[c]) - (hi - lo), -1,
                                           np.int64)])
        idx = np.arange(len(L))
        col_g = idx // (N_CORES * 128)
        core = (idx % (N_CORES * 128)) // 128
        p = idx % 128
        s = s_cum[c] // 128 + col_g
        l_slot = s * 128 + p
        row_at_slot_g[core, l_slot] = L
        real = L >= 0
        pos_of_global[L[real]] = (NW + N_CORES * s_cum[c]
                                  + core[real] * (s_cum[c + 1] - s_cum[c])
                                  + col_g[real] * 128 + p[real])
        # per-global-column deps (same for every core)
        for name, cls in (("xy", cx), ("yz", cy)):
            cls_L = np.where(real, cls[np.where(real, L, 0)], -2)
            for cg in range(int(cap[c]) // (N_CORES * 128)):
                span = cls_L[cg * N_CORES * 128:(cg + 1) * N_CORES * 128]
                plan_cols[name][s_cum[c] // 128 + cg] = span.max()

    # rewritten per-core per-slot gather indices + mask
    isx_np = np.zeros((N_CORES, 128, PS), np.int32)
    isy_np = np.zeros((N_CORES, 128, PS), np.int32)
    msk_np = np.zeros((N_CORES, 128, PS), np.int8)
    for k in range(N_CORES):
        ras = row_at_slot_g[k]
        valid = ras >= 0
        g = np.where(valid, ras, 0)
        m_slot = np.where(valid, mask_a[g], False)

        def rewrite(idx_w):
            i0 = idx_w[g]
            a = np.maximum(i0 - NW, 0)
            out = np.where(i0 < NW, i0, pos_of_global[a])
            return np.where(m_slot, out, 0).astype(np.int32)

        isx_np[k] = rewrite(ixy_w).reshape(PS, 128).T
        isy_np[k] = rewrite(iyz_w).reshape(PS, 128).T
        msk_np[k] = m_slot.astype(np.int8).reshape(PS, 128).T

    gather_plan = []
    for name in ("xy", "yz"):
        for s in range(PS):
            d = int(plan_cols[name][s])
            if d == -2:
                continue                         # no slot needs this column
            ub = NW if d == -1 else NW + N_CORES * int(s_cum[d + 1])
            gather_plan.append((ub, name, s))
    gather_plan.sort(key=lambda e: e[0])

    plan_key = hashlib.sha256(repr(gather_plan).encode()).hexdigest()
    if _CACHE.get("plan_key") != plan_key:
        _CACHE["nc"] = _build(gather_plan)
        _CACHE["plan_key"] = plan_key
    nc = _CACHE["nc"]

    # ---------- numeric inputs ----------
    w1bf = np.ascontiguousarray(
        np.asarray(W1, np.float32).reshape(DC, 128, H)).astype(ml_dtypes.bfloat16)
    w2bf = np.ascontiguousarray(
        np.asarray(W2, np.float32).reshape(HC, 128, 3)).astype(ml_dtypes.bfloat16)
    b1f = np.asarray(b1, np.float32)
    b2r = np.asarray(b2, np.float32).reshape(3, 1)
    win3 = np.ascontiguousarray(within_pre[:, :3], np.float32)
    cwb = np.broadcast_to(
        np.asarray(clause_weights, np.float32).reshape(1, N_LAYERS * 3),
        (128, N_LAYERS * 3)).copy()

    in_maps = []
    for k in range(N_CORES):
        ras = row_at_slot_g[k]
        src = np.where(ras >= 0, ras, 0)
        fp = features[src].astype(ml_dtypes.bfloat16)
        fp[ras < 0] = 0
        featT_k = np.ascontiguousarray(fp.T.reshape(DC, 128, RPAD))
        in_maps.append({
            "featT": featT_k,
            "w1b": w1bf, "w2b": w2bf, "b1": b1f, "b2": b2r,
            "win3": win3, "cw": cwb,
            "isx": isx_np[k], "isy": isy_np[k], "pmask": msk_np[k],
        })

    res = run_bass_kernel_spmd(nc, in_maps, core_ids=list(range(N_CORES)))
    _CACHE["last_results"] = res

    out = np.empty((NA, 3), np.float32)
    smx = np.empty((NA, 3), np.float32)
    for k in range(N_CORES):
        raw_o = res.results[k]["out_o"].reshape(128, PS, 3)
        raw_s = res.results[k]["sm_o"].reshape(128, PS, 3)
        o_slot = raw_o.transpose(1, 0, 2).reshape(RPAD, 3)
        s_slot = raw_s.transpose(1, 0, 2).reshape(RPAD, 3)
        ras = row_at_slot_g[k]
        valid = ras >= 0
        out[ras[valid]] = o_slot[valid]
        smx[ras[valid]] = s_slot[valid]
    return out, smx


# revision 13
# speedup vs baseline: 1.9036x; 1.0337x over previous
"""Trainium2 Bass kernel for nn_Kenn_across (gnn_message_passing).

Pipeline on 8 NeuronCores (SPMD), per core:
  MLP     across = relu(featT @ W1 + b1) @ W2 + b2 on RPAD=25088 rows,
          features host-pretransposed to bf16 [DC,128,RPAD] (no PE transposes).
  AG      chunked AllGather of across slices into the replicated pre table.
          Only chunks 0..2 are AllGathered: the last chunk holds exactly the
          across rows that NO xy/yz gather references (~26% of rows), so it
          needs no collective and the whole gather+KENN pipeline for the
          first chunks hides under the last chunk's MLP window.
  gather  per-column [128,1] indirect DMAs for u_xy / u_yz, issued in
          dependency order (within-table columns at t=0, chunk-c columns
          right after AG_c). u_xz needs NO gather: each across row's winning
          triple has index_xz == that row, so u_xz == own across value.
  KENN    3 clause-enhancement layers + masked select + softmax, split into
          group A (chunks 0-2 columns, emitted mid-MLP so it overlaps the
          last-chunk matmuls) and group B (last-chunk columns, after MLP).

Host-side prep (integer bookkeeping + feature permute/bf16):
  - winner permutation per across row (last-write-wins scatter semantics)
  - referenced-set computation; unreferenced rows dealt to the last chunk
  - global column-synchronized class sort: rows sorted by (xy-dep, yz-dep)
    within each chunk and dealt across cores column-by-column, so gather
    columns have identical dependency classes on every core; mask-0 slots
    sort first and their columns are skipped entirely
  - gather indices rewritten to the device table layout
"""
import hashlib
import numpy as np
import ml_dtypes

import concourse.bass as bass
import concourse.mybir as mybir
import concourse.tile as tile
from concourse import bacc
from concourse.bass_utils import run_bass_kernel_spmd

# problem constants (spec: nn_Kenn_across_29661044146692)
N_CORES = 8
NW, NA, M = 100000, 200000, 262144
D, H = 1024, 1024
RPC = NA // N_CORES              # 25000 across rows per core
PS = 196                         # columns (slots per partition)
RPAD = 128 * PS                  # 25088 padded slots per core
PADS = N_CORES * RPAD - NA       # 704 global pad slots (in the last chunk)
DC = D // 128
HC = H // 128
IT = 512                         # MLP tile rows; 49 * 512 == RPAD
NTILES = RPAD // IT
N_LAYERS = 3
NCH = 4                          # chunks; last one is not AllGathered
KENN_A_TILE = 46                 # emit group-A KENN after this MLP tile

f32 = mybir.dt.float32
bf16 = mybir.dt.bfloat16
i32 = mybir.dt.int32
i8 = mybir.dt.int8
AF = mybir.ActivationFunctionType
ALU = mybir.AluOpType

_CACHE = {}


def _build(gather_plan, sc_cols):
    """gather_plan: sorted list of (ub_rows, name, col); the indirect gather
    for that column reads pre[0:ub_rows]. sc_cols: columns per chunk."""
    s_cum = [0]
    for c in sc_cols:
        s_cum.append(s_cum[-1] + c * 128)
    assert s_cum[-1] == RPAD
    nt3 = NW + N_CORES * s_cum[NCH - 1]       # pre rows (last chunk excluded)
    ag_tiles = {s_cum[c + 1] // IT - 1: c for c in range(NCH - 1)}
    cut_ubs = [NW] + [NW + N_CORES * s_cum[c + 1] for c in range(NCH - 1)]
    cuts = [sum(1 for ub, _, _ in gather_plan if ub <= u) for u in cut_ubs]
    c012 = s_cum[NCH - 1] // 128              # group-A columns [0, c012)

    nc = bacc.Bacc("TRN2", target_bir_lowering=False, debug=False,
                   num_devices=N_CORES)

    featT = nc.dram_tensor("featT", [DC, 128, RPAD], bf16, kind="ExternalInput")
    w1b = nc.dram_tensor("w1b", [DC, 128, H], bf16, kind="ExternalInput")
    w2b = nc.dram_tensor("w2b", [HC, 128, 3], bf16, kind="ExternalInput")
    b1 = nc.dram_tensor("b1", [H], f32, kind="ExternalInput")
    b2 = nc.dram_tensor("b2", [3, 1], f32, kind="ExternalInput")
    win3 = nc.dram_tensor("win3", [NW, 3], f32, kind="ExternalInput")
    cw = nc.dram_tensor("cw", [128, N_LAYERS * 3], f32, kind="ExternalInput")
    isx = nc.dram_tensor("isx", [128, PS], i32, kind="ExternalInput")
    isy = nc.dram_tensor("isy", [128, PS], i32, kind="ExternalInput")
    pmask = nc.dram_tensor("pmask", [128, PS], i8, kind="ExternalInput")

    out_o = nc.dram_tensor("out_o", [128, PS * 3], f32, kind="ExternalOutput")
    sm_o = nc.dram_tensor("sm_o", [128, PS * 3], f32, kind="ExternalOutput")

    with tile.TileContext(nc) as tc:
        with tc.tile_pool(name="const", bufs=1) as cp, \
             tc.tile_pool(name="work", bufs=2) as wp, \
             tc.tile_pool(name="ps1", bufs=2, space="PSUM") as pp1, \
             tc.tile_pool(name="ps2", bufs=2, space="PSUM") as pp2, \
             tc.tile_pool(name="dram", bufs=1, space="DRAM") as dp:

            # ---------- constants ----------
            w1sb = cp.tile([128, DC, H], bf16)
            nc.sync.dma_start(w1sb[:], w1b[:].rearrange("d p h -> p d h"))
            w2sb = cp.tile([128, HC, 3], bf16)
            nc.sync.dma_start(w2sb[:], w2b[:].rearrange("h p c -> p h c"))
            b1sb = cp.tile([128, HC], f32)
            nc.sync.dma_start(b1sb[:], b1[:].rearrange("(hc p) -> p hc", p=128))
            b2sb = cp.tile([3, 1], f32)
            nc.sync.dma_start(b2sb[:], b2[:])
            cwsb = cp.tile([128, 1, N_LAYERS * 3], f32)
            nc.sync.dma_start(cwsb[:], cw[:])
            isxsb = cp.tile([128, PS], i32)
            nc.sync.dma_start(isxsb[:], isx[:])
            isysb = cp.tile([128, PS], i32)
            nc.sync.dma_start(isysb[:], isy[:])
            msb = cp.tile([128, PS, 1], i8)
            nc.sync.dma_start(msb[:], pmask[:])

            pre = dp.tile([nt3, 3], f32)
            acc = dp.tile([RPAD, 3], f32)
            agt = []
            for c in range(NCH - 1):
                agt_c = dp.tile([N_CORES * (s_cum[c + 1] - s_cum[c]), 3], f32,
                                addr_space="Shared", name=f"agt{c}")
                agt.append(agt_c)
            nc.sync.dma_start(pre[0:NW, :], win3[:])

            uxy = wp.tile([128, PS, 3], f32, tag="uxy", bufs=1)
            uyz = wp.tile([128, PS, 3], f32, tag="uyz", bufs=1)
            uxz = wp.tile([128, PS, 3], f32, tag="uxz", bufs=1)
            acr = wp.tile([128, PS, 3], f32, tag="acr", bufs=1)
            nc.vector.memset(uxy[:], 0.0)
            nc.vector.memset(uyz[:], 0.0)

            def issue_gathers(lo, hi):
                for ub, name, s in gather_plan[lo:hi]:
                    t_ = uxy if name == "xy" else uyz
                    sb_ = isxsb if name == "xy" else isysb
                    nc.gpsimd.indirect_dma_start(
                        out=t_[:, s, :], out_offset=None, in_=pre[0:ub, :],
                        in_offset=bass.IndirectOffsetOnAxis(
                            ap=sb_[:, s:s + 1], axis=0))

            u = {"xy": uxy, "yz": uyz, "xz": uxz}

            def kenn_block(lo, hi, g):
                w = hi - lo
                for l in range(N_LAYERS):
                    exy = wp.tile([128, w, 3], f32, tag=f"exy{g}", name=f"exy{g}")
                    eyz = wp.tile([128, w, 3], f32, tag=f"eyz{g}", name=f"eyz{g}")
                    exz = wp.tile([128, w, 3], f32, tag=f"exz{g}", name=f"exz{g}")
                    nc.scalar.activation(exy[:], u["xy"][:, lo:hi, :], AF.Exp,
                                         scale=-1.0)
                    nc.scalar.activation(eyz[:], u["yz"][:, lo:hi, :], AF.Exp,
                                         scale=-1.0)
                    nc.scalar.activation(exz[:], u["xz"][:, lo:hi, :], AF.Exp,
                                         scale=1.0)
                    ssum = wp.tile([128, w, 3], f32, tag=f"ssum{g}",
                                   name=f"ssum{g}")
                    nc.vector.tensor_tensor(ssum[:], exy[:], eyz[:], op=ALU.add)
                    nc.vector.tensor_tensor(ssum[:], ssum[:], exz[:], op=ALU.add)
                    nc.vector.reciprocal(ssum[:], ssum[:])
                    rw = wp.tile([128, w, 3], f32, tag=f"rw{g}", name=f"rw{g}")
                    cwb = cwsb[:, :, l * 3:(l + 1) * 3].to_broadcast([128, w, 3])
                    nc.vector.tensor_tensor(rw[:], ssum[:], cwb, op=ALU.mult)
                    for name, op in (("xy", ALU.subtract), ("yz", ALU.subtract),
                                     ("xz", ALU.add)):
                        e = {"xy": exy, "yz": eyz, "xz": exz}[name]
                        nc.vector.tensor_tensor(e[:], e[:], rw[:], op=ALU.mult)
                        nc.vector.tensor_tensor(u[name][:, lo:hi, :],
                                                u[name][:, lo:hi, :], e[:], op=op)
                # masked select + softmax + outputs for this column range
                nc.vector.copy_predicated(
                    acr[:, lo:hi, :],
                    msb[:, lo:hi, :].to_broadcast([128, w, 3]),
                    u["xz"][:, lo:hi, :])
                a3 = acr[:, lo:hi, :]
                mx = wp.tile([128, w], f32, tag=f"mx{g}", name=f"mx{g}")
                nc.vector.tensor_tensor(mx[:], a3[:, :, 0], a3[:, :, 1],
                                        op=ALU.max)
                nc.vector.tensor_tensor(mx[:], mx[:], a3[:, :, 2], op=ALU.max)
                e3 = wp.tile([128, w, 3], f32, tag=f"e3{g}", name=f"e3{g}")
                for cc in range(3):
                    nc.vector.tensor_tensor(e3[:, :, cc], a3[:, :, cc], mx[:],
                                            op=ALU.subtract)
                nc.scalar.activation(e3[:], e3[:], AF.Exp, scale=1.0)
                ssm = wp.tile([128, w], f32, tag=f"ssm{g}", name=f"ssm{g}")
                nc.vector.tensor_tensor(ssm[:], e3[:, :, 0], e3[:, :, 1],
                                        op=ALU.add)
                nc.vector.tensor_tensor(ssm[:], ssm[:], e3[:, :, 2], op=ALU.add)
                nc.vector.reciprocal(ssm[:], ssm[:])
                sm = wp.tile([128, w, 3], f32, tag=f"sm{g}", name=f"sm{g}")
                for cc in range(3):
                    nc.vector.tensor_tensor(sm[:, :, cc], e3[:, :, cc], ssm[:],
                                            op=ALU.mult)
                nc.sync.dma_start(out_o[:, lo * 3:hi * 3],
                                  acr[:, lo:hi, :].rearrange("p s c -> p (s c)"))
                nc.sync.dma_start(sm_o[:, lo * 3:hi * 3],
                                  sm[:].rearrange("p s c -> p (s c)"))

            issue_gathers(0, cuts[0])          # within-table columns

            # ---------- MLP ----------
            for t in range(NTILES):
                r0 = t * IT
                at = wp.tile([128, DC, IT], bf16, tag="at", bufs=3)
                nc.sync.dma_start(
                    at[:], featT[:, :, r0:r0 + IT].rearrange("d p r -> p d r"))
                c1t = wp.tile([128, HC, IT], bf16, tag="c1t")
                for hc in range(HC):
                    p1 = pp1.tile([128, IT], f32, tag="p1")
                    for dc in range(DC):
                        nc.tensor.matmul(p1[:],
                                         lhsT=w1sb[:, dc, hc * 128:(hc + 1) * 128],
                                         rhs=at[:, dc, :],
                                         start=(dc == 0), stop=(dc == DC - 1))
                    nc.scalar.activation(c1t[:, hc, :], p1[:], AF.Relu,
                                         bias=b1sb[:, hc:hc + 1], scale=1.0)
                p2 = pp2.tile([3, IT], f32, tag="p2")
                for hc in range(HC):
                    nc.tensor.matmul(p2[:], lhsT=w2sb[:, hc, :],
                                     rhs=c1t[:, hc, :],
                                     start=(hc == 0), stop=(hc == HC - 1))
                acc_sb = wp.tile([3, IT], f32, tag="acc", bufs=3)
                nc.scalar.activation(acc_sb[:], p2[:], AF.Identity,
                                     bias=b2sb[:, 0:1], scale=1.0)
                nc.sync.dma_start(
                    acc[r0:r0 + IT, :].rearrange("r c -> c r"), acc_sb[:])

                if t in ag_tiles:
                    c = ag_tiles[t]
                    nc.gpsimd.collective_compute(
                        "AllGather", ALU.bypass,
                        replica_groups=[list(range(N_CORES))],
                        ins=[acc[s_cum[c]:s_cum[c + 1], :]],
                        outs=[agt[c][:]])
                    nc.sync.dma_start(
                        pre[NW + N_CORES * s_cum[c]:
                            NW + N_CORES * s_cum[c + 1], :], agt[c][:])
                    issue_gathers(cuts[c], cuts[c + 1])

                if t == KENN_A_TILE:
                    # group A: chunks 0-2 columns; deps (gathers + acc[0:S3])
                    # are met well before this point in the schedule
                    nc.sync.dma_start(
                        uxz[:, 0:c012, :],
                        acc[0:s_cum[NCH - 1], :].rearrange(
                            "(s p) c -> p s c", p=128))
                    nc.sync.dma_start(
                        acr[:, 0:c012, :],
                        acc[0:s_cum[NCH - 1], :].rearrange(
                            "(s p) c -> p s c", p=128))
                    kenn_block(0, c012, "a")

            # ---------- group B: last-chunk columns ----------
            nc.sync.dma_start(
                uxz[:, c012:PS, :],
                acc[s_cum[NCH - 1]:RPAD, :].rearrange("(s p) c -> p s c", p=128))
            nc.sync.dma_start(
                acr[:, c012:PS, :],
                acc[s_cum[NCH - 1]:RPAD, :].rearrange("(s p) c -> p s c", p=128))
            kenn_block(c012, PS, "b")

    nc.compile()
    return nc


def kernel(features, within_pre, index_xy, index_yz, index_xz,
           W1, b1, W2, b2, clause_weights):
    features = np.asarray(features)
    within_pre = np.asarray(within_pre)
    index_xy = np.asarray(index_xy, np.int64)
    index_yz = np.asarray(index_yz, np.int64)
    index_xz = np.asarray(index_xz, np.int64)

    NT0 = NW + NA
    # winner per across row (numpy last-write-wins scatter semantics)
    perm_full = np.full(NT0, -1, np.int64)
    perm_full[index_xz] = np.arange(M)
    perm_a = perm_full[NW:]
    mask_a = perm_a >= 0
    pidx_a = np.where(mask_a, perm_a, 0)
    ixy_w = index_xy[pidx_a]          # per global across row j: winner's xy idx
    iyz_w = index_yz[pidx_a]

    # referenced across rows (by any mask-1 slot's xy/yz)
    referenced = np.zeros(NA, bool)
    for idx_w in (ixy_w, iyz_w):
        tgt = idx_w[mask_a]
        tgt = tgt[tgt >= NW] - NW
        referenced[tgt] = True
    unref_ids = np.nonzero(~referenced)[0]
    ref_ids = np.nonzero(referenced)[0]

    # chunk geometry: last chunk = unreferenced rows only (no AllGather)
    cols3 = min(((len(unref_ids) + PADS) // 1024) // 4 * 4, 64)
    cols3 = max(cols3, 0)
    cap3_rows = cols3 * 1024 - PADS
    rem = PS - cols3
    c0 = (rem // 3) // 4 * 4
    c1 = (rem // 3) // 4 * 4
    sc_cols = (c0, c1, rem - c0 - c1, cols3)
    s_cum = np.concatenate([[0], np.cumsum([c * 128 for c in sc_cols])])
    cap = np.asarray([N_CORES * 128 * c for c in sc_cols], np.int64)

    # fixed chunk assignment (pre-sort): referenced rows (plus unreferenced
    # spill) deal sequentially into chunks 0..2; the rest fill chunk 3
    chunk3_rows = unref_ids[:cap3_rows]
    relike = np.sort(np.concatenate([ref_ids, unref_ids[cap3_rows:]]))
    chunk_of_row = np.empty(NA, np.int64)
    chunk_of_row[chunk3_rows] = NCH - 1
    b0 = int(cap[0])
    b1_ = b0 + int(cap[1])
    chunk_of_row[relike[:b0]] = 0
    chunk_of_row[relike[b0:b1_]] = 1
    chunk_of_row[relike[b1_:]] = 2
    row_lists = [relike[:b0], relike[b0:b1_], relike[b1_:], chunk3_rows]

    # dep class per global across row for each name:
    #   -2 no need (mask 0), -1 within-table, 0..2 referenced AG chunk
    def dep_class(idx_w):
        is_w = idx_w < NW
        cls = np.where(is_w, -1, chunk_of_row[np.maximum(idx_w - NW, 0)])
        return np.where(mask_a, cls, -2).astype(np.int64)

    cx = dep_class(ixy_w)
    cy = dep_class(iyz_w)
    assert cx.max() <= NCH - 2 and cy.max() <= NCH - 2

    # global within-chunk sort by (cx, cy), column-synchronized deal
    key_all = (cx + 2) * (NCH + 2) + (cy + 2)
    row_at_slot_g = np.full((N_CORES, RPAD), -1, np.int64)  # global row ids
    pos_of_global = np.full(NA, -1, np.int64)               # device table row
    plan_cols = {"xy": np.full(PS, -2, np.int64),
                 "yz": np.full(PS, -2, np.int64)}
    for c in range(NCH):
        rows_c = row_lists[c]
        order = rows_c[np.argsort(key_all[rows_c], kind="stable")]
        L = np.concatenate([order, np.full(int(cap[c]) - len(rows_c), -1,
                                           np.int64)])
        idx = np.arange(len(L))
        col_g = idx // (N_CORES * 128)
        core = (idx % (N_CORES * 128)) // 128
        p = idx % 128
        s = s_cum[c] // 128 + col_g
        l_slot = s * 128 + p
        row_at_slot_g[core, l_slot] = L
        real = L >= 0
        pos_of_global[L[real]] = (NW + N_CORES * s_cum[c]
                                  + core[real] * (s_cum[c + 1] - s_cum[c])
                                  + col_g[real] * 128 + p[real])
        for name, cls in (("xy", cx), ("yz", cy)):
            cls_L = np.where(real, cls[np.where(real, L, 0)], -2)
            for cg in range(int(cap[c]) // (N_CORES * 128)):
                span = cls_L[cg * N_CORES * 128:(cg + 1) * N_CORES * 128]
                plan_cols[name][s_cum[c] // 128 + cg] = span.max()

    # rewritten per-core per-slot gather indices + mask
    isx_np = np.zeros((N_CORES, 128, PS), np.int32)
    isy_np = np.zeros((N_CORES, 128, PS), np.int32)
    msk_np = np.zeros((N_CORES, 128, PS), np.int8)
    for k in range(N_CORES):
        ras = row_at_slot_g[k]
        valid = ras >= 0
        g = np.where(valid, ras, 0)
        m_slot = np.where(valid, mask_a[g], False)

        def rewrite(idx_w):
            i0 = idx_w[g]
            a = np.maximum(i0 - NW, 0)
            out = np.where(i0 < NW, i0, pos_of_global[a])
            return np.where(m_slot, out, 0).astype(np.int32)

        isx_np[k] = rewrite(ixy_w).reshape(PS, 128).T
        isy_np[k] = rewrite(iyz_w).reshape(PS, 128).T
        msk_np[k] = m_slot.astype(np.int8).reshape(PS, 128).T
    assert isx_np.max() < NW + N_CORES * int(s_cum[NCH - 1])
    assert isy_np.max() < NW + N_CORES * int(s_cum[NCH - 1])

    gather_plan = []
    for name in ("xy", "yz"):
        for s in range(PS):
            d = int(plan_cols[name][s])
            if d == -2:
                continue                         # no slot needs this column
            ub = NW if d == -1 else NW + N_CORES * int(s_cum[d + 1])
            gather_plan.append((ub, name, s))
    gather_plan.sort(key=lambda e: e[0])

    plan_key = hashlib.sha256(
        (repr(gather_plan) + repr(sc_cols)).encode()).hexdigest()
    if _CACHE.get("plan_key") != plan_key:
        _CACHE["nc"] = _build(gather_plan, sc_cols)
        _CACHE["plan_key"] = plan_key
    nc = _CACHE["nc"]

    # ---------- numeric inputs ----------
    w1bf = np.ascontiguousarray(
        np.asarray(W1, np.float32).reshape(DC, 128, H)).astype(ml_dtypes.bfloat16)
    w2bf = np.ascontiguousarray(
        np.asarray(W2, np.float32).reshape(HC, 128, 3)).astype(ml_dtypes.bfloat16)
    b1f = np.asarray(b1, np.float32)
    b2r = np.asarray(b2, np.float32).reshape(3, 1)
    win3 = np.ascontiguousarray(within_pre[:, :3], np.float32)
    cwb = np.broadcast_to(
        np.asarray(clause_weights, np.float32).reshape(1, N_LAYERS * 3),
        (128, N_LAYERS * 3)).copy()

    in_maps = []
    for k in range(N_CORES):
        ras = row_at_slot_g[k]
        src = np.where(ras >= 0, ras, 0)
        fp = features[src].astype(ml_dtypes.bfloat16)
        fp[ras < 0] = 0
        featT_k = np.ascontiguousarray(fp.T.reshape(DC, 128, RPAD))
        in_maps.append({
            "featT": featT_k,
            "w1b": w1bf, "w2b": w2bf, "b1": b1f, "b2": b2r,
            "win3": win3, "cw": cwb,
            "isx": isx_np[k], "isy": isy_np[k], "pmask": msk_np[k],
        })

    res = run_bass_kernel_spmd(nc, in_maps, core_ids=list(range(N_CORES)))
    _CACHE["last_results"] = res

    out = np.empty((NA, 3), np.float32)
    smx = np.empty((NA, 3), np.float32)
    for k in range(N_CORES):
        raw_o = res.results[k]["out_o"].reshape(128, PS, 3)
        raw_s = res.results[k]["sm_o"].reshape(128, PS, 3)
        o_slot = raw_o.transpose(1, 0, 2).reshape(RPAD, 3)
        s_slot = raw_s.transpose(1, 0, 2).reshape(RPAD, 3)
        ras = row_at_slot_g[k]
        valid = ras >= 0
        out[ras[valid]] = o_slot[valid]
        smx[ras[valid]] = s_slot[valid]
    return out, smx
